# revision 3
# baseline (speedup 1.0000x reference)
"""GraphUNet on Trainium2: 8-core SPMD, 4 device launches.

Structure (launch boundaries forced by host top-k selections):
  L1: GCN0 aggregation  aggT = y0^T @ Aeff0^T  (row-sharded, fp8 adjacency)
  L2: M0 = Ap0[keep0,:] @ Ap0[:,keep0]  (fp8 DoubleRow integer matmul, the
      dominant compute; 4x2 2D output sharding) fused with the GCN1
      aggregation M0 @ y1 (column-partials summed on host).
  L3: up1 aggregation  C0eff @ y_up1  (row-sharded)
  L4: up2 aggregation  Aeff0 @ y_up2  (row-sharded, same NEFF as L1)

The level-2/3 subtree (x2, x3, xup0) has magnitudes 3.6e-6 / 4.5e-13 /
1.5e-6 vs final output absmax 0.16 — it is computed exactly on host in
fp32 (cheap: ~4.6 GFLOP), feeding the device up-path.

Numerics: adjacency-valued matrices are small exact integers (A<=3,
M0<=7) -> fp8 e4m3 exact (DoubleRow runs fp8 at 2x). Feature vectors are
hi/lo fp16 splits in columns 0:16 / 32:48 (32-aligned so the PSUM halves
can be read separately). All matmul accumulation is fp32 in PSUM.

Device arrays are staged partition-major ([128, ch*w]) on the host so
each DMA is 128 fat contiguous descriptors.
"""
import os
import numpy as np
import ml_dtypes

import concourse.bass as bass
import concourse.mybir as mybir
import concourse.tile as tile
from concourse import bacc
from concourse.bass_utils import run_bass_kernel_spmd

N, E, F, D = 4096, 131072, 14, 16
NCORE = 8
K0, K1, K2 = 2048, 1024, 512
RB, CB = 512, 1024       # L2 2D shard: 4 row-blocks x 2 col-blocks

fp8 = ml_dtypes.float8_e4m3

MOCK = os.environ.get("BASS_MOCK") == "1"

_cache = {}
_launch_log = []


def _split48(y):
    """[n,16] f32 -> [n,48] fp16 with hi at 0:16, lo at 32:48."""
    n = y.shape[0]
    out = np.zeros((n, 48), np.float16)
    h = y.astype(np.float16)
    out[:, 0:16] = h
    out[:, 32:48] = (y - h.astype(np.float32)).astype(np.float16)
    return out


def _pmajor(a):
    """[ch*128, w] -> [128, ch*w] partition-major staging for fat DMAs."""
    n, w = a.shape
    ch = n // 128
    return np.ascontiguousarray(
        a.reshape(ch, 128, w).transpose(1, 0, 2).reshape(128, ch * w))


def build_agg_neff(n, w):
    """aggT[48, w] = sum_c y[c]^T @ at[c]  over n/128 chunks.
    y: [128, ch*48] fp16 partition-major, at: [128, ch*w] fp8 p-major."""
    key = ("agg", n, w)
    if key in _cache:
        return _cache[key]
    ch = n // 128
    G = 2
    nc = bacc.Bacc("TRN2", target_bir_lowering=False, debug=False)
    y = nc.dram_tensor("y", [128, ch * 48], mybir.dt.float16, kind="ExternalInput")
    at = nc.dram_tensor("at", [128, ch * w], mybir.dt.float8e4, kind="ExternalInput")
    out = nc.dram_tensor("out", [48, w], mybir.dt.float32, kind="ExternalOutput")
    with tile.TileContext(nc) as tc:
        with (
            tc.tile_pool(name="sb", bufs=1) as sb,
            tc.tile_pool(name="st", bufs=8) as st,
            tc.tile_pool(name="ps", bufs=1, space="PSUM") as ps,
        ):
            yt = sb.tile([128, ch, 48], mybir.dt.float16)
            nc.scalar.dma_start(yt[:], y[:, :])
            pt = ps.tile([48, w], mybir.dt.float32)
            for g in range(ch // G):
                eng = nc.sync if g % 2 == 0 else nc.scalar
                att = st.tile([128, G, w], mybir.dt.float8e4, name="att", tag="att")
                eng.dma_start(att[:], at[:, g * G * w:(g + 1) * G * w])
                for c in range(G):
                    cc = g * G + c
                    nc.tensor.matmul(pt[:], lhsT=yt[:, cc, :], rhs=att[:, c, :],
                                     start=(cc == 0), stop=(cc == ch - 1))
            o = sb.tile([48, w], mybir.dt.float32)
            nc.vector.tensor_copy(o[:], pt[:])
            nc.sync.dma_start(out[:], o[:])
    nc.compile()
    _cache[key] = nc
    return nc


def build_m0_neff():
    """Per-core 2D block of M0 = Ap0[keep0,:] @ Ap0[:,keep0] + fused partial
    GCN1 aggregation.

    lt: Ap0^T[:, keep0[rowblk]] [N, RB] fp8 (own M0 output rows)
    rh: Ap0[:, keep0[colblk]]   [N, CB] fp8 (own M0 output cols)
    y1: [CB, 48] fp16 (hi/lo split, colblk slice)
    out m0t: [CB, RB] fp8 = M0^T[colblk, rowblk];  agg: [RB, 48] f32 partial.
    """
    key = ("m0",)
    if key in _cache:
        return _cache[key]
    CH = N // 128            # 32 contraction chunks
    JT = CB // 128           # 8
    CG = 512                 # rh column-group streamed per DMA
    JTL = CG // 128
    nc = bacc.Bacc("TRN2", target_bir_lowering=False, debug=False)
    lt = nc.dram_tensor("lt", [N, RB], mybir.dt.float8e4, kind="ExternalInput")
    rh = nc.dram_tensor("rh", [N, CB], mybir.dt.float8e4, kind="ExternalInput")
    y1 = nc.dram_tensor("y1", [128, JT * 48], mybir.dt.float16, kind="ExternalInput")
    m0t = nc.dram_tensor("m0t", [CB, RB], mybir.dt.float8e4, kind="ExternalOutput")
    agg = nc.dram_tensor("agg", [RB, 48], mybir.dt.float32, kind="ExternalOutput")
    with tile.TileContext(nc) as tc:
        with (
            tc.tile_pool(name="sb", bufs=1) as sb,
            tc.tile_pool(name="st", bufs=2) as st,
            tc.tile_pool(name="sm", bufs=4) as sm,
            tc.tile_pool(name="ps", bufs=2, space="PSUM") as ps,
            tc.tile_pool(name="pa", bufs=1, space="PSUM") as pa,
        ):
            ltt = sb.tile([128, CH, RB], mybir.dt.float8e4)
            nc.scalar.dma_start(ltt[:], lt.rearrange("(c p) d -> p c d", p=128))
            y1t = sb.tile([128, JT, 48], mybir.dt.float16)
            nc.scalar.dma_start(y1t[:], y1[:, :])
            aggp = [pa.tile([128, 48], mybir.dt.float32, name=f"aggp{i}",
                            tag=f"aggp{i}") for i in range(RB // 128)]
            for g in range(CB // CG):
                rhg = st.tile([128, CH, CG], mybir.dt.float8e4, name="rhg", tag="rhg")
                nc.sync.dma_start(rhg[:], rh[:, g * CG:(g + 1) * CG].rearrange(
                    "(c p) d -> p c d", p=128))
                for j in range(JTL):
                    jt = g * JTL + j
                    psT = ps.tile([128, RB], mybir.dt.float32, name="psT", tag="psT")
                    for kc in range(0, CH, 2):
                        nc.tensor.matmul(
                            psT[:], lhsT=rhg[:, kc:kc + 2, j * 128:(j + 1) * 128],
                            rhs=ltt[:, kc:kc + 2, :],
                            perf_mode=mybir.MatmulPerfMode.DoubleRow,
                            start=(kc == 0), stop=(kc == CH - 2))
                    m8 = sm.tile([128, RB], mybir.dt.float8e4, name="m8", tag="m8")
                    nc.vector.tensor_copy(m8[:], psT[:])
                    nc.sync.dma_start(m0t[jt * 128:(jt + 1) * 128, :], m8[:])
                    for it in range(RB // 128):
                        nc.tensor.matmul(aggp[it][:],
                                         lhsT=m8[:, it * 128:(it + 1) * 128],
                                         rhs=y1t[:, jt, :],
                                         start=(jt == 0), stop=(jt == JT - 1))
            for it in range(RB // 128):
                ot = sb.tile([128, 48], mybir.dt.float32, name=f"ot{it}",
                             tag=f"ot{it}")
                nc.vector.tensor_copy(ot[:], aggp[it][:])
                nc.sync.dma_start(agg[it * 128:(it + 1) * 128, :], ot[:])
    nc.compile()
    _cache[key] = nc
    return nc


def _run(nc, in_maps, name):
    import time as _t
    t0 = _t.time()
    if MOCK:
        res = _mock_run(nc, in_maps)
    else:
        r = run_bass_kernel_spmd(nc, in_maps, core_ids=list(range(NCORE)),
                                 trace=False)
        res = r.results
    _launch_log.append((name, _t.time() - t0))
    return res


def _unpmajor(a, ch, w):
    return a.reshape(128, ch, w).transpose(1, 0, 2).reshape(ch * 128, w)


def _mock_run(nc, in_maps):
    outs = []
    for m in in_maps:
        if "y" in m:   # agg neff: p-major inputs
            ch = m["y"].shape[1] // 48
            w = m["at"].shape[1] // ch
            y = _unpmajor(m["y"], ch, 48).astype(np.float32)
            at = _unpmajor(m["at"], ch, w).astype(np.float32)
            outs.append({"out": y.T @ at})
        else:          # m0 neff
            lt = m["lt"].astype(np.float32)
            rh = m["rh"].astype(np.float32)
            y1 = _unpmajor(m["y1"], CB // 128, 48).astype(np.float32)
            m0t_f = rh.T @ lt
            m0t = m0t_f.astype(fp8)
            agg = m0t.astype(np.float32).T @ y1
            outs.append({"m0t": m0t, "agg": agg})
    return outs


def _agg_launch(n, w_total, ycat, at_T_fp8, name):
    """Row-sharded aggregation: agg [w_total, 16] f32 = (adj @ y)."""
    w = w_total // NCORE
    nc = build_agg_neff(n, w)
    ypm = _pmajor(ycat)
    in_maps = []
    for c in range(NCORE):
        sl = slice(c * w, (c + 1) * w)
        in_maps.append({"y": ypm, "at": _pmajor(at_T_fp8[:, sl])})
    outs = _run(nc, in_maps, name)
    aggT = np.concatenate([o["out"] for o in outs], axis=1)
    return (aggT[0:16] + aggT[32:48]).T


def kernel(**inputs):
    x = np.asarray(inputs["x"], np.float32)
    ei = np.asarray(inputs["edge_index"]).astype(np.int64)
    W = {k: np.asarray(v, np.float32) for k, v in inputs.items()
         if k not in ("x", "edge_index")}

    # ---- host prep: dense adjacency
    idx = ei[0] * N + ei[1]
    A = np.bincount(idx, minlength=N * N).astype(np.float32).reshape(N, N)
    d0 = np.diagonal(A).copy()
    Aeff0 = A.copy()
    np.fill_diagonal(Aeff0, d0 + (d0 == 0))
    deg0 = Aeff0.sum(1)
    dis0 = 1.0 / np.sqrt(deg0)
    Aeff0T8 = np.ascontiguousarray(Aeff0.T).astype(fp8)
    Ap0 = A.copy()
    np.fill_diagonal(Ap0, 1.0)          # offdiag(A) + I
    Ap0_8 = Ap0.astype(fp8)
    Ap0T8 = np.ascontiguousarray(Ap0.T).astype(fp8)

    # ---- L1: GCN0 aggregation on device
    y0 = dis0[:, None] * (x @ W["W_d0"])
    x0 = _agg_launch(N, N, _split48(y0), Aeff0T8, "L1-gcn0")
    x0 = np.maximum(dis0[:, None] * x0, 0.0)

    # ---- top-k level 0 (host)
    p0 = W["p0"]
    s0 = np.tanh((x0 @ p0) / np.linalg.norm(p0))
    keep0 = np.sort(np.argsort(-s0, kind="stable")[:K0])
    vals0 = s0[keep0]

    # deg/diag of M0 via matvecs (exact, cheap)
    L0g = Ap0[keep0, :]
    R0g = Ap0[:, keep0]
    rs1 = L0g @ R0g.sum(axis=1)
    diagM0 = np.einsum('ij,ji->i', L0g, R0g)
    deg1 = rs1 - diagM0 + 1.0
    dis1 = 1.0 / np.sqrt(deg1)

    x0p = x0[keep0] * vals0[:, None]
    y1 = dis1[:, None] * (x0p @ W["W_d1"])
    y1cat = _split48(y1)

    # ---- L2: M0 matmul + GCN1 aggregation on device (4x2 2D shard)
    nc = build_m0_neff()
    in_maps = []
    for c in range(NCORE):
        r, cc = c // 2, c % 2
        rows = keep0[r * RB:(r + 1) * RB]
        cols = keep0[cc * CB:(cc + 1) * CB]
        in_maps.append({
            "lt": np.ascontiguousarray(Ap0T8[:, rows]),
            "rh": np.ascontiguousarray(Ap0_8[:, cols]),
            "y1": _pmajor(y1cat[cc * CB:(cc + 1) * CB]),
        })
    outs = _run(nc, in_maps, "L2-m0")
    M0T = np.empty((K0, K0), fp8)
    agg1 = np.zeros((K0, 48), np.float32)
    for c in range(NCORE):
        r, cc = c // 2, c % 2
        M0T[cc * CB:(cc + 1) * CB, r * RB:(r + 1) * RB] = outs[c]["m0t"]
        agg1[r * RB:(r + 1) * RB] += outs[c]["agg"]
    agg1 = agg1[:, 0:16] + agg1[:, 32:48]
    # diag correction: device used raw M0 (diag=diagM0), want diag=1
    agg1 = agg1 + (1.0 - diagM0)[:, None] * y1
    x1 = np.maximum(dis1[:, None] * agg1, 0.0)

    # ---- levels 2..3 subtree on host (magnitudes ~1e-6 of output scale,
    #      computed exactly in fp32)
    M0 = M0T.astype(np.float32).T
    C0eff = M0.copy()
    np.fill_diagonal(C0eff, 1.0)

    p1 = W["p1"]
    s1 = np.tanh((x1 @ p1) / np.linalg.norm(p1))
    keep1 = np.sort(np.argsort(-s1, kind="stable")[:K1])
    vals1 = s1[keep1]
    M1 = C0eff[keep1, :] @ C0eff[:, keep1]
    M1eff = M1.copy()
    np.fill_diagonal(M1eff, 1.0)
    deg2 = M1eff.sum(1)
    dis2 = 1.0 / np.sqrt(deg2)
    x1p = x1[keep1] * vals1[:, None]
    x2 = np.maximum(dis2[:, None] * (M1eff @ (dis2[:, None] * (x1p @ W["W_d2"]))), 0.0)

    p2 = W["p2"]
    s2 = np.tanh((x2 @ p2) / np.linalg.norm(p2))
    keep2 = np.sort(np.argsort(-s2, kind="stable")[:K2])
    vals2 = s2[keep2]
    M2 = M1eff[keep2, :] @ M1eff[:, keep2]
    M2eff = M2.copy()
    np.fill_diagonal(M2eff, 1.0)
    deg3 = M2eff.sum(1)
    dis3 = 1.0 / np.sqrt(deg3)
    x2p = x2[keep2] * vals2[:, None]
    x3 = np.maximum(dis3[:, None] * (M2eff @ (dis3[:, None] * (x2p @ W["W_d3"]))), 0.0)

    # up0 (host): adjacency = M1eff, nodes K1
    xu2 = x2.copy()
    xu2[keep2] += x3
    xup0 = np.maximum(
        dis2[:, None] * (M1eff @ (dis2[:, None] * (xu2 @ W["W_u0"]))), 0.0)

    # ---- L3: up1 aggregation on device (adjacency C0eff, nodes K0)
    xu1 = x1.copy()
    xu1[keep1] += xup0
    y_up1 = dis1[:, None] * (xu1 @ W["W_u1"])
    C0effT8 = np.ascontiguousarray(M0T)
    ii = np.arange(K0)
    C0effT8[ii, ii] = fp8(1.0)
    agg_up1 = _agg_launch(K0, K0, _split48(y_up1), C0effT8, "L3-up1")
    xup1 = np.maximum(dis1[:, None] * agg_up1, 0.0)

    # ---- L4: up2 aggregation on device (adjacency Aeff0, nodes N)
    xu0 = x0.copy()
    xu0[keep0] += xup1
    y_up2 = dis0[:, None] * (xu0 @ W["W_u2"])
    agg_up2 = _agg_launch(N, N, _split48(y_up2), Aeff0T8, "L4-up2")
    out = dis0[:, None] * agg_up2
    return out.astype(np.float32)


# revision 6
# speedup vs baseline: 1.2300x; 1.2300x over previous
"""GraphUNet on Trainium2: 8-core SPMD, 4 device launches.

Structure (launch boundaries forced by host top-k selections):
  L1: GCN0 aggregation  aggT = y0^T @ Aeff0^T  (row-sharded, fp8 adjacency)
  L2: M0 = Ap0[keep0,:] @ Ap0[:,keep0]  (fp8 DoubleRow integer matmul, the
      dominant compute; 4x2 2D output sharding) fused with the GCN1
      aggregation M0 @ y1 (column-partials summed on host).
  L3: up1 aggregation  C0eff @ y_up1  (row-sharded)
  L4: up2 aggregation  Aeff0 @ y_up2  (row-sharded, same NEFF as L1)

The level-2/3 subtree (x2, x3, xup0) has magnitudes 3.6e-6 / 4.5e-13 /
1.5e-6 vs final output absmax 0.16 — it is computed exactly on host in
fp32 (cheap: ~4.6 GFLOP), feeding the device up-path.

Numerics: adjacency-valued matrices are small exact integers (A<=3,
M0<=7) -> fp8 e4m3 exact (DoubleRow runs fp8 at 2x). Feature vectors are
hi/lo fp16 splits in columns 0:16 / 32:48 (32-aligned so the PSUM halves
can be read separately). All matmul accumulation is fp32 in PSUM.

Device arrays are staged partition-major ([128, ch*w]) on the host so
each DMA is 128 fat contiguous descriptors.
"""
import os
import numpy as np
import ml_dtypes

import concourse.bass as bass
import concourse.mybir as mybir
import concourse.tile as tile
from concourse import bacc
from concourse.bass_utils import run_bass_kernel_spmd

N, E, F, D = 4096, 131072, 14, 16
NCORE = 8
K0, K1, K2 = 2048, 1024, 512
RB, CB = 512, 1024       # L2 2D shard: 4 row-blocks x 2 col-blocks

fp8 = ml_dtypes.float8_e4m3

MOCK = os.environ.get("BASS_MOCK") == "1"

_cache = {}
_launch_log = []


def _split48(y):
    """[n,16] f32 -> [n,48] fp16 with hi at 0:16, lo at 32:48."""
    n = y.shape[0]
    out = np.zeros((n, 48), np.float16)
    h = y.astype(np.float16)
    out[:, 0:16] = h
    out[:, 32:48] = (y - h.astype(np.float32)).astype(np.float16)
    return out


def _split80(y):
    """[n,16] f32 -> [n,80] fp8: h@0:16, (y-h)*64@32:48, residual*4096@64:80."""
    n = y.shape[0]
    out = np.zeros((n, 80), fp8)
    h = y.astype(fp8)
    hf = h.astype(np.float32)
    m = ((y - hf) * 64).astype(fp8)
    mf = m.astype(np.float32) / 64
    l = ((y - hf - mf) * 4096).astype(fp8)
    out[:, 0:16] = h
    out[:, 32:48] = m
    out[:, 64:80] = l
    return out


def _pmajor(a):
    """[ch*128, w] -> [128, ch*w] partition-major staging for fat DMAs."""
    n, w = a.shape
    ch = n // 128
    return np.ascontiguousarray(
        a.reshape(ch, 128, w).transpose(1, 0, 2).reshape(128, ch * w))


def build_agg_neff(n, w):
    """aggT[16, w] = adj-aggregation via fp8 DoubleRow.
    y: [128, ch*80] fp8 p-major (h@0:16, m*64@32:48, l*4096@64:80),
    at: [128, ch*w] fp8 p-major (adjacency^T columns)."""
    key = ("agg", n, w)
    if key in _cache:
        return _cache[key]
    ch = n // 128
    G = 4
    nc = bacc.Bacc("TRN2", target_bir_lowering=False, debug=False)
    y = nc.dram_tensor("y", [128, ch * 80], mybir.dt.float8e4, kind="ExternalInput")
    at = nc.dram_tensor("at", [128, ch * w], mybir.dt.float8e4, kind="ExternalInput")
    out = nc.dram_tensor("out", [16, w], mybir.dt.float32, kind="ExternalOutput")
    with tile.TileContext(nc) as tc:
        with (
            tc.tile_pool(name="sb", bufs=1) as sb,
            tc.tile_pool(name="st", bufs=8) as st,
            tc.tile_pool(name="ps", bufs=1, space="PSUM") as ps,
        ):
            yt = sb.tile([128, ch, 80], mybir.dt.float8e4)
            nc.scalar.dma_start(yt[:], y[:, :])
            pt = ps.tile([80, w], mybir.dt.float32)
            for g in range(ch // G):
                eng = nc.sync if g % 2 == 0 else nc.scalar
                att = st.tile([128, G, w], mybir.dt.float8e4, name="att", tag="att")
                eng.dma_start(att[:], at[:, g * G * w:(g + 1) * G * w])
                for cp in range(0, G, 2):
                    cc = g * G + cp
                    nc.tensor.matmul(pt[:], lhsT=yt[:, cc:cc + 2, :],
                                     rhs=att[:, cp:cp + 2, :],
                                     perf_mode=mybir.MatmulPerfMode.DoubleRow,
                                     start=(cc == 0), stop=(cc == ch - 2))
            o = sb.tile([16, w], mybir.dt.float32)
            nc.vector.tensor_copy(o[:], pt[0:16, :])
            nc.vector.scalar_tensor_tensor(o[:], pt[32:48, :], 1.0 / 64, o[:],
                                           op0=mybir.AluOpType.mult,
                                           op1=mybir.AluOpType.add)
            nc.vector.scalar_tensor_tensor(o[:], pt[64:80, :], 1.0 / 4096, o[:],
                                           op0=mybir.AluOpType.mult,
                                           op1=mybir.AluOpType.add)
            nc.sync.dma_start(out[:], o[:])
    nc.compile()
    _cache[key] = nc
    return nc


def build_m0_neff():
    """Per-core 2D block of M0 = Ap0[keep0,:] @ Ap0[:,keep0] + fused partial
    GCN1 aggregation.

    lt: Ap0^T[:, keep0[rowblk]] [N, RB] fp8 (own M0 output rows)
    rh: Ap0[:, keep0[colblk]]   [N, CB] fp8 (own M0 output cols)
    y1: [CB, 48] fp16 (hi/lo split, colblk slice)
    out m0t: [CB, RB] fp8 = M0^T[colblk, rowblk];  agg: [RB, 48] f32 partial.
    """
    key = ("m0",)
    if key in _cache:
        return _cache[key]
    CH = N // 128            # 32 contraction chunks
    JT = CB // 128           # 8
    CG = 512                 # rh column-group streamed per DMA set
    JTL = CG // 128
    LTS = 4                  # lt contraction-split DMAs
    RHS = 8                  # rh contraction-split DMAs per group
    nc = bacc.Bacc("TRN2", target_bir_lowering=False, debug=False)
    lt = nc.dram_tensor("lt", [N, RB], mybir.dt.float8e4, kind="ExternalInput")
    rh = nc.dram_tensor("rh", [N, CB], mybir.dt.float8e4, kind="ExternalInput")
    y1 = nc.dram_tensor("y1", [128, JT * 48], mybir.dt.float16, kind="ExternalInput")
    m0t = nc.dram_tensor("m0t", [CB, RB], mybir.dt.float8e4, kind="ExternalOutput")
    agg = nc.dram_tensor("agg", [RB, 48], mybir.dt.float32, kind="ExternalOutput")
    with tile.TileContext(nc) as tc:
        with (
            tc.tile_pool(name="sb", bufs=1) as sb,
            tc.tile_pool(name="st", bufs=3) as st,
            tc.tile_pool(name="sm", bufs=4) as sm,
            tc.tile_pool(name="ps", bufs=2, space="PSUM") as ps,
            tc.tile_pool(name="pa", bufs=1, space="PSUM") as pa,
        ):
            ltt = sb.tile([128, CH, RB], mybir.dt.float8e4)
            HS = CH // LTS
            for h in range(LTS):
                nc.scalar.dma_start(
                    ltt[:, h * HS:(h + 1) * HS, :],
                    lt[h * HS * 128:(h + 1) * HS * 128, :].rearrange(
                        "(c p) d -> p c d", p=128))
            y1t = sb.tile([128, JT, 48], mybir.dt.float16)
            nc.scalar.dma_start(y1t[:], y1[:, :])
            aggp = [pa.tile([128, 48], mybir.dt.float32, name=f"aggp{i}",
                            tag=f"aggp{i}") for i in range(RB // 128)]
            RS = CH // RHS
            for g in range(CB // CG):
                rhg = st.tile([128, CH, CG], mybir.dt.float8e4, name="rhg", tag="rhg")
                for h in range(RHS):
                    nc.sync.dma_start(
                        rhg[:, h * RS:(h + 1) * RS, :],
                        rh[h * RS * 128:(h + 1) * RS * 128,
                           g * CG:(g + 1) * CG].rearrange("(c p) d -> p c d", p=128))
                for j in range(JTL):
                    jt = g * JTL + j
                    psT = ps.tile([128, RB], mybir.dt.float32, name="psT", tag="psT")
                    for kc in range(0, CH, 2):
                        nc.tensor.matmul(
                            psT[:], lhsT=rhg[:, kc:kc + 2, j * 128:(j + 1) * 128],
                            rhs=ltt[:, kc:kc + 2, :],
                            perf_mode=mybir.MatmulPerfMode.DoubleRow,
                            start=(kc == 0), stop=(kc == CH - 2))
                    m8 = sm.tile([128, RB], mybir.dt.float8e4, name="m8", tag="m8")
                    nc.vector.tensor_copy(m8[:], psT[:])
                    nc.scalar.dma_start(m0t[jt * 128:(jt + 1) * 128, :], m8[:])
                    for it in range(RB // 128):
                        nc.tensor.matmul(aggp[it][:],
                                         lhsT=m8[:, it * 128:(it + 1) * 128],
                                         rhs=y1t[:, jt, :],
                                         start=(jt == 0), stop=(jt == JT - 1))
            for it in range(RB // 128):
                ot = sb.tile([128, 48], mybir.dt.float32, name=f"ot{it}",
                             tag=f"ot{it}")
                nc.vector.tensor_copy(ot[:], aggp[it][:])
                nc.scalar.dma_start(agg[it * 128:(it + 1) * 128, :], ot[:])
    nc.compile()
    _cache[key] = nc
    return nc


def _run(nc, in_maps, name):
    import time as _t
    t0 = _t.time()
    if MOCK:
        res = _mock_run(nc, in_maps)
    else:
        r = run_bass_kernel_spmd(nc, in_maps, core_ids=list(range(NCORE)),
                                 trace=False)
        res = r.results
    _launch_log.append((name, _t.time() - t0))
    return res


def _unpmajor(a, ch, w):
    return a.reshape(128, ch, w).transpose(1, 0, 2).reshape(ch * 128, w)


def _mock_run(nc, in_maps):
    outs = []
    for m in in_maps:
        if "y" in m:   # agg neff: p-major fp8 triple-split inputs
            ch = m["y"].shape[1] // 80
            w = m["at"].shape[1] // ch
            y80 = _unpmajor(m["y"], ch, 80).astype(np.float32)
            y = y80[:, 0:16] + y80[:, 32:48] / 64 + y80[:, 64:80] / 4096
            at = _unpmajor(m["at"], ch, w).astype(np.float32)
            outs.append({"out": y.T @ at})
        else:          # m0 neff
            lt = m["lt"].astype(np.float32)
            rh = m["rh"].astype(np.float32)
            y1 = _unpmajor(m["y1"], CB // 128, 48).astype(np.float32)
            m0t_f = rh.T @ lt
            m0t = m0t_f.astype(fp8)
            agg = m0t.astype(np.float32).T @ y1
            outs.append({"m0t": m0t, "agg": agg})
    return outs


def _agg_launch(n, w_total, y, at_T_fp8, name):
    """Row-sharded aggregation: agg [w_total, 16] f32 = (adj @ y)."""
    w = w_total // NCORE
    nc = build_agg_neff(n, w)
    ypm = _pmajor(_split80(y))
    in_maps = []
    for c in range(NCORE):
        sl = slice(c * w, (c + 1) * w)
        in_maps.append({"y": ypm, "at": _pmajor(at_T_fp8[:, sl])})
    outs = _run(nc, in_maps, name)
    aggT = np.concatenate([o["out"] for o in outs], axis=1)
    return aggT.T


def kernel(**inputs):
    x = np.asarray(inputs["x"], np.float32)
    ei = np.asarray(inputs["edge_index"]).astype(np.int64)
    W = {k: np.asarray(v, np.float32) for k, v in inputs.items()
         if k not in ("x", "edge_index")}

    # ---- host prep: dense adjacency
    idx = ei[0] * N + ei[1]
    A = np.bincount(idx, minlength=N * N).astype(np.float32).reshape(N, N)
    d0 = np.diagonal(A).copy()
    Aeff0 = A.copy()
    np.fill_diagonal(Aeff0, d0 + (d0 == 0))
    deg0 = Aeff0.sum(1)
    dis0 = 1.0 / np.sqrt(deg0)
    Aeff0T8 = np.ascontiguousarray(Aeff0.T).astype(fp8)
    Ap0 = A.copy()
    np.fill_diagonal(Ap0, 1.0)          # offdiag(A) + I
    Ap0_8 = Ap0.astype(fp8)
    Ap0T8 = np.ascontiguousarray(Ap0.T).astype(fp8)

    # ---- L1: GCN0 aggregation on device
    y0 = dis0[:, None] * (x @ W["W_d0"])
    x0 = _agg_launch(N, N, y0, Aeff0T8, "L1-gcn0")
    x0 = np.maximum(dis0[:, None] * x0, 0.0)

    # ---- top-k level 0 (host)
    p0 = W["p0"]
    s0 = np.tanh((x0 @ p0) / np.linalg.norm(p0))
    keep0 = np.sort(np.argsort(-s0, kind="stable")[:K0])
    vals0 = s0[keep0]

    # deg/diag of M0 via matvecs (exact, cheap)
    L0g = Ap0[keep0, :]
    R0g = Ap0[:, keep0]
    rs1 = L0g @ R0g.sum(axis=1)
    diagM0 = np.einsum('ij,ji->i', L0g, R0g)
    deg1 = rs1 - diagM0 + 1.0
    dis1 = 1.0 / np.sqrt(deg1)

    x0p = x0[keep0] * vals0[:, None]
    y1 = dis1[:, None] * (x0p @ W["W_d1"])
    y1cat = _split48(y1)

    # ---- L2: M0 matmul + GCN1 aggregation on device (4x2 2D shard)
    nc = build_m0_neff()
    in_maps = []
    for c in range(NCORE):
        r, cc = c // 2, c % 2
        rows = keep0[r * RB:(r + 1) * RB]
        cols = keep0[cc * CB:(cc + 1) * CB]
        in_maps.append({
            "lt": np.ascontiguousarray(Ap0T8[:, rows]),
            "rh": np.ascontiguousarray(Ap0_8[:, cols]),
            "y1": _pmajor(y1cat[cc * CB:(cc + 1) * CB]),
        })
    outs = _run(nc, in_maps, "L2-m0")
    M0T = np.empty((K0, K0), fp8)
    agg1 = np.zeros((K0, 48), np.float32)
    for c in range(NCORE):
        r, cc = c // 2, c % 2
        M0T[cc * CB:(cc + 1) * CB, r * RB:(r + 1) * RB] = outs[c]["m0t"]
        agg1[r * RB:(r + 1) * RB] += outs[c]["agg"]
    agg1 = agg1[:, 0:16] + agg1[:, 32:48]
    # diag correction: device used raw M0 (diag=diagM0), want diag=1
    agg1 = agg1 + (1.0 - diagM0)[:, None] * y1
    x1 = np.maximum(dis1[:, None] * agg1, 0.0)

    # ---- levels 2..3 subtree on host (magnitudes ~1e-6 of output scale,
    #      computed exactly in fp32)
    M0 = M0T.astype(np.float32).T
    C0eff = M0.copy()
    np.fill_diagonal(C0eff, 1.0)

    p1 = W["p1"]
    s1 = np.tanh((x1 @ p1) / np.linalg.norm(p1))
    keep1 = np.sort(np.argsort(-s1, kind="stable")[:K1])
    vals1 = s1[keep1]
    M1 = C0eff[keep1, :] @ C0eff[:, keep1]
    M1eff = M1.copy()
    np.fill_diagonal(M1eff, 1.0)
    deg2 = M1eff.sum(1)
    dis2 = 1.0 / np.sqrt(deg2)
    x1p = x1[keep1] * vals1[:, None]
    x2 = np.maximum(dis2[:, None] * (M1eff @ (dis2[:, None] * (x1p @ W["W_d2"]))), 0.0)

    p2 = W["p2"]
    s2 = np.tanh((x2 @ p2) / np.linalg.norm(p2))
    keep2 = np.sort(np.argsort(-s2, kind="stable")[:K2])
    vals2 = s2[keep2]
    M2 = M1eff[keep2, :] @ M1eff[:, keep2]
    M2eff = M2.copy()
    np.fill_diagonal(M2eff, 1.0)
    deg3 = M2eff.sum(1)
    dis3 = 1.0 / np.sqrt(deg3)
    x2p = x2[keep2] * vals2[:, None]
    x3 = np.maximum(dis3[:, None] * (M2eff @ (dis3[:, None] * (x2p @ W["W_d3"]))), 0.0)

    # up0 (host): adjacency = M1eff, nodes K1
    xu2 = x2.copy()
    xu2[keep2] += x3
    xup0 = np.maximum(
        dis2[:, None] * (M1eff @ (dis2[:, None] * (xu2 @ W["W_u0"]))), 0.0)

    # ---- L3: up1 aggregation on device (adjacency C0eff, nodes K0)
    xu1 = x1.copy()
    xu1[keep1] += xup0
    y_up1 = dis1[:, None] * (xu1 @ W["W_u1"])
    C0effT8 = np.ascontiguousarray(M0T)
    ii = np.arange(K0)
    C0effT8[ii, ii] = fp8(1.0)
    agg_up1 = _agg_launch(K0, K0, y_up1, C0effT8, "L3-up1")
    xup1 = np.maximum(dis1[:, None] * agg_up1, 0.0)

    # ---- L4: up2 aggregation on device (adjacency Aeff0, nodes N)
    xu0 = x0.copy()
    xu0[keep0] += xup1
    y_up2 = dis0[:, None] * (xu0 @ W["W_u2"])
    agg_up2 = _agg_launch(N, N, y_up2, Aeff0T8, "L4-up2")
    out = dis0[:, None] * agg_up2
    return out.astype(np.float32)
